# revision 15
# baseline (speedup 1.0000x reference)
"""BlockSparseLinearWithPerm Trainium2 kernel.

Math: out[b,s,j] = sum_i x[b,s,in_perm[i]] * W[out_perm[j], i] + bias[out_perm[j]]
where W is the dense form of the block-sparse weight.

Both permutations and the block scatter are folded on the host into a dense
effective weight  W_effT[k, j] = sum_{i: in_perm[i]==k} W[out_perm[j], i]
(host cost: one 1024x1024 scatter-add — negligible), so the device kernel is a
pure dense matmul  out = x @ W_effT + bias_eff, data-parallel over the batch
dim: one batch element (8192x1024 tokens) per NeuronCore, weights replicated.

Sharding/layout: each core's x slice is shipped feature-major (x^T) so the
contraction dim lands on SBUF partitions directly — the device spends zero
TensorE cycles on transposes and runs a pure f32r matmul stream (full
1 cycle/row PE rate, tf32-class mantissa, ~1e-3 scale-relative error).
Per 128-token tile: 16 accumulating f32r matmuls (lhsT = x^T k-tiles,
moving = resident W_effT) -> VectorE adds bias while copying PSUM -> SBUF
-> DMA out in natural token-major layout.
"""
import os
import sys
import subprocess
import tempfile

import numpy as np

_TRN_REPO = "/opt/trn_rl_repo"

D_IN = 1024
D_OUT = 1024
BS = 64
R = D_OUT // BS
C = D_IN // BS
BATCH = 8
SEQ = 8192
P = 128
KT = D_IN // P          # 8 k-tiles
WIN = 1024              # tokens per x^T window
NWIN = SEQ // WIN       # 8 windows
N_CORES = 8


def _fold_weights(weight_blocks, brow, bcol, bias, in_perm, out_perm):
    """Fold block scatter + both permutations into W_effT [k, j] and bias_eff."""
    wb = np.asarray(weight_blocks, dtype=np.float64)
    brow = np.asarray(brow).astype(np.int64)
    bcol = np.asarray(bcol).astype(np.int64)
    in_perm = np.asarray(in_perm).astype(np.int64)
    out_perm = np.asarray(out_perm).astype(np.int64)
    W4 = np.zeros((R, C, BS, BS), dtype=np.float64)
    W4[brow, bcol] = wb
    W = W4.transpose(0, 2, 1, 3).reshape(D_OUT, D_IN)
    Wp = W[out_perm]                       # [j, i]
    W_effT = np.zeros((D_IN, D_OUT), dtype=np.float64)
    np.add.at(W_effT, in_perm, Wp.T)       # row i of Wp.T added into row in_perm[i]
    bias_eff = np.asarray(bias, dtype=np.float64)[out_perm]
    bias_bcast = np.broadcast_to(bias_eff, (P, D_OUT)).copy()
    return (np.ascontiguousarray(W_effT, dtype=np.float64).astype(np.float32),
            bias_bcast.astype(np.float32))


_NC_CACHE = {}


def _build_nc():
    if "nc" in _NC_CACHE:
        return _NC_CACHE["nc"]
    if _TRN_REPO not in sys.path:
        sys.path.insert(0, _TRN_REPO)
    import concourse.bacc as bacc
    import concourse.mybir as mybir
    from concourse.tile import TileContext
    from contextlib import ExitStack

    F32 = mybir.dt.float32
    F32R = mybir.dt.float32r

    nc = bacc.Bacc(target_bir_lowering=False)
    xt_d = nc.declare_dram_parameter("xt", [D_IN, SEQ], F32R, isOutput=False)
    wt_d = nc.declare_dram_parameter("wt", [D_IN, D_OUT], F32R, isOutput=False)
    bias_d = nc.declare_dram_parameter("bias", [P, D_OUT], F32, isOutput=False)
    out_d = nc.declare_dram_parameter("out", [SEQ, D_OUT], F32, isOutput=True)

    xt_r = xt_d.rearrange("(kt p) s -> p kt s", p=P)
    wt_r = wt_d.rearrange("(kt p) j -> p kt j", p=P)

    with TileContext(nc) as tc, ExitStack() as ctx:
        consts = ctx.enter_context(tc.tile_pool(name="consts", bufs=1))
        xpool = ctx.enter_context(tc.tile_pool(name="xpool", bufs=2))
        opool = ctx.enter_context(tc.tile_pool(name="opool", bufs=9))
        ps_o = ctx.enter_context(tc.tile_pool(name="ps_o", bufs=4, space="PSUM"))

        bias_sb = consts.tile([P, D_OUT], F32)
        # W is split per (k-tile, j-half); all jh=0 halves stream FIRST on the
        # scalar ring so the first window's jh0 chains only wait on 2MB of W.
        w_tiles = [[consts.tile([P, 512], F32R, name=f"w_{kt}_{jh}")
                    for jh in range(2)] for kt in range(KT)]

        xwin0 = xpool.tile([P, KT, WIN], F32R, tag="xw", name="xwin")
        for kt in range(KT):
            nc.sync.dma_start(out=xwin0[:, kt], in_=xt_r[:, kt, 0:WIN])
        for jh in range(2):
            for kt in range(KT):
                nc.scalar.dma_start(
                    out=w_tiles[kt][jh],
                    in_=wt_r[:, kt, jh * 512:(jh + 1) * 512])
        nc.scalar.dma_start(out=bias_sb, in_=bias_d[:, :])

        def mm_chain(xwin, s_lo, po, jh):
            for kt in range(KT):
                nc.tensor.matmul(
                    po, xwin[:, kt, s_lo:s_lo + P], w_tiles[kt][jh],
                    start=(kt == 0), stop=(kt == KT - 1))

        def bias_out(out_sb, po, jh):
            nc.vector.tensor_add(
                out=out_sb[:, jh * 512:(jh + 1) * 512],
                in0=po, in1=bias_sb[:, jh * 512:(jh + 1) * 512])

        for win in range(NWIN):
            if win == 0:
                xwin = xwin0
                # jh-major over the whole window: the 8 jh0 chains run as soon
                # as the first 2MB of W lands, jh1 chains follow.
                outs = [opool.tile([P, D_OUT], F32, tag="o", name="out_sb")
                        for _ in range(WIN // P)]
                for jh in range(2):
                    for ss in range(WIN // P):
                        po = ps_o.tile([P, 512], F32, tag=f"po{jh}",
                                       name=f"po{jh}")
                        mm_chain(xwin, ss * P, po, jh)
                        bias_out(outs[ss], po, jh)
                for ss in range(WIN // P):
                    nc.scalar.dma_start(
                        out=out_d[ss * P:(ss + 1) * P, :], in_=outs[ss])
                continue
            xwin = xpool.tile([P, KT, WIN], F32R, tag="xw", name="xwin")
            nc.sync.dma_start(
                out=xwin, in_=xt_r[:, :, win * WIN:(win + 1) * WIN])
            for ss in range(WIN // P):
                s_lo = ss * P
                out_sb = opool.tile([P, D_OUT], F32, tag="o", name="out_sb")
                pos = [ps_o.tile([P, 512], F32, tag=f"po{jh}", name=f"po{jh}")
                       for jh in range(2)]
                for jh in range(2):
                    mm_chain(xwin, s_lo, pos[jh], jh)
                for jh in range(2):
                    bias_out(out_sb, pos[jh], jh)
                st = win * (WIN // P) + ss
                nc.scalar.dma_start(
                    out=out_d[st * P:(st + 1) * P, :], in_=out_sb)

    nc.finalize()
    _NC_CACHE["nc"] = nc
    return nc


def _run_device(x, W_effT, bias_bcast, trace=False, tmpdir=None):
    """Run the SPMD kernel on 8 cores in this process. Returns (out, exec_ns)."""
    if _TRN_REPO not in sys.path:
        sys.path.insert(0, _TRN_REPO)
    from concourse.bass_utils import run_bass_kernel_spmd

    nc = _build_nc()
    core_ids = list(range(N_CORES))
    in_maps = [
        {"xt": np.ascontiguousarray(np.asarray(x[c], dtype=np.float32).T),
         "wt": W_effT, "bias": bias_bcast}
        for c in core_ids
    ]
    res = run_bass_kernel_spmd(nc, in_maps, core_ids, trace=trace, tmpdir=tmpdir)
    out = np.stack([res.results[c]["out"] for c in core_ids], axis=0)
    return out, res.exec_time_ns


def _kernel_impl(x, in_perm, out_perm, weight_blocks, brow, bcol, bias,
                 trace=False, tmpdir=None):
    x = np.asarray(x)
    W_effT, bias_bcast = _fold_weights(
        weight_blocks, brow, bcol, bias, in_perm, out_perm)
    out, exec_ns = _run_device(
        x.reshape(BATCH, SEQ, D_IN), W_effT, bias_bcast,
        trace=trace, tmpdir=tmpdir)
    return out.astype(np.float32), exec_ns


def _axon_usable_inproc():
    """True if this process can (still) drive the axon trn2 backend.
    If the caller pinned JAX_PLATFORMS to something without axon, importing
    jax here would initialize the wrong backend — run in a subprocess
    instead (and never half-initialize an axon client we can't use)."""
    jp = os.environ.get("JAX_PLATFORMS", "")
    if jp and "axon" not in jp:
        return False
    try:
        import jax
        return any(d.platform == "axon" for d in jax.devices())
    except Exception:
        return False


def kernel(x, in_perm, out_perm, weight_blocks, brow, bcol, bias):
    if _axon_usable_inproc():
        try:
            out, _ = _kernel_impl(
                x, in_perm, out_perm, weight_blocks, brow, bcol, bias)
            return out
        except Exception:
            pass
    # Fallback: run the device part in a clean subprocess (e.g. if the
    # calling process pinned JAX_PLATFORMS=cpu before importing jax).
    return _kernel_subprocess(
        x, in_perm, out_perm, weight_blocks, brow, bcol, bias)


def _kernel_subprocess(x, in_perm, out_perm, weight_blocks, brow, bcol, bias):
    with tempfile.TemporaryDirectory() as td:
        inp = os.path.join(td, "in.npz")
        outp = os.path.join(td, "out.npy")
        np.savez(inp, x=x, in_perm=in_perm, out_perm=out_perm,
                 weight_blocks=weight_blocks, brow=brow, bcol=bcol, bias=bias)
        env = dict(os.environ)
        env.pop("JAX_PLATFORMS", None)
        last_err = None
        for attempt in range(3):
            try:
                subprocess.run(
                    [sys.executable, os.path.abspath(__file__),
                     "--serve", inp, outp],
                    check=True, env=env)
                return np.load(outp)
            except subprocess.CalledProcessError as e:
                last_err = e
                import time
                time.sleep(15)
        raise last_err


if __name__ == "__main__":
    if len(sys.argv) == 4 and sys.argv[1] == "--serve":
        data = np.load(sys.argv[2])
        out, _ = _kernel_impl(
            data["x"], data["in_perm"], data["out_perm"],
            data["weight_blocks"], data["brow"], data["bcol"], data["bias"])
        np.save(sys.argv[3], out)


# revision 16
# speedup vs baseline: 1.0388x; 1.0388x over previous
"""BlockSparseLinearWithPerm Trainium2 kernel.

Math: out[b,s,j] = sum_i x[b,s,in_perm[i]] * W[out_perm[j], i] + bias[out_perm[j]]
where W is the dense form of the block-sparse weight.

Both permutations and the block scatter are folded on the host into a dense
effective weight  W_effT[k, j] = sum_{i: in_perm[i]==k} W[out_perm[j], i]
(host cost: one 1024x1024 scatter-add — negligible), so the device kernel is a
pure dense matmul  out = x @ W_effT + bias_eff, data-parallel over the batch
dim: one batch element (8192x1024 tokens) per NeuronCore, weights replicated.

Sharding/layout: each core's x slice is shipped feature-major (x^T) so the
contraction dim lands on SBUF partitions directly — the device spends zero
TensorE cycles on transposes and runs a pure f32r matmul stream (full
1 cycle/row PE rate, tf32-class mantissa, ~1e-3 scale-relative error).
Per 128-token tile: 16 accumulating f32r matmuls (lhsT = x^T k-tiles,
moving = resident W_effT) -> VectorE adds bias while copying PSUM -> SBUF
-> DMA out in natural token-major layout.
"""
import os
import sys
import subprocess
import tempfile

import numpy as np

_TRN_REPO = "/opt/trn_rl_repo"

D_IN = 1024
D_OUT = 1024
BS = 64
R = D_OUT // BS
C = D_IN // BS
BATCH = 8
SEQ = 8192
P = 128
KT = D_IN // P          # 8 k-tiles
WIN = 1024              # tokens per x^T window
NWIN = SEQ // WIN       # 8 windows
N_CORES = 8


def _fold_weights(weight_blocks, brow, bcol, bias, in_perm, out_perm):
    """Fold block scatter + both permutations into W_effT [k, j] and bias_eff."""
    wb = np.asarray(weight_blocks, dtype=np.float64)
    brow = np.asarray(brow).astype(np.int64)
    bcol = np.asarray(bcol).astype(np.int64)
    in_perm = np.asarray(in_perm).astype(np.int64)
    out_perm = np.asarray(out_perm).astype(np.int64)
    W4 = np.zeros((R, C, BS, BS), dtype=np.float64)
    W4[brow, bcol] = wb
    W = W4.transpose(0, 2, 1, 3).reshape(D_OUT, D_IN)
    Wp = W[out_perm]                       # [j, i]
    W_effT = np.zeros((D_IN, D_OUT), dtype=np.float64)
    np.add.at(W_effT, in_perm, Wp.T)       # row i of Wp.T added into row in_perm[i]
    bias_eff = np.asarray(bias, dtype=np.float64)[out_perm]
    bias_bcast = np.broadcast_to(bias_eff, (P, D_OUT)).copy()
    return (np.ascontiguousarray(W_effT, dtype=np.float64).astype(np.float32),
            bias_bcast.astype(np.float32))


_NC_CACHE = {}


def _build_nc():
    if "nc" in _NC_CACHE:
        return _NC_CACHE["nc"]
    if _TRN_REPO not in sys.path:
        sys.path.insert(0, _TRN_REPO)
    import concourse.bacc as bacc
    import concourse.mybir as mybir
    from concourse.tile import TileContext
    from contextlib import ExitStack

    F32 = mybir.dt.float32
    F32R = mybir.dt.float32r

    nc = bacc.Bacc(target_bir_lowering=False)
    xt_d = nc.declare_dram_parameter("xt", [D_IN, SEQ], F32R, isOutput=False)
    wt_d = nc.declare_dram_parameter("wt", [D_IN, D_OUT], F32R, isOutput=False)
    bias_d = nc.declare_dram_parameter("bias", [P, D_OUT], F32, isOutput=False)
    out_d = nc.declare_dram_parameter("out", [SEQ, D_OUT], F32, isOutput=True)

    xt_r = xt_d.rearrange("(kt p) s -> p kt s", p=P)
    wt_r = wt_d.rearrange("(kt p) j -> p kt j", p=P)

    with TileContext(nc) as tc, ExitStack() as ctx:
        consts = ctx.enter_context(tc.tile_pool(name="consts", bufs=1))
        xpool = ctx.enter_context(tc.tile_pool(name="xpool", bufs=2))
        opool = ctx.enter_context(tc.tile_pool(name="opool", bufs=4))
        ps_o = ctx.enter_context(tc.tile_pool(name="ps_o", bufs=4, space="PSUM"))

        bias_sb = consts.tile([P, D_OUT], F32)
        # Per-kt W tiles; x window chunks + W interleave across the two HWDGE
        # rings so the first matmul chains unblock early.
        w_tiles = [consts.tile([P, D_OUT], F32R, name=f"w_{kt}")
                   for kt in range(KT)]

        xwin0 = xpool.tile([P, KT, WIN], F32R, tag="xw", name="xwin")
        for kt in range(KT):
            nc.sync.dma_start(out=xwin0[:, kt], in_=xt_r[:, kt, 0:WIN])
            nc.scalar.dma_start(out=w_tiles[kt], in_=wt_r[:, kt])
        nc.scalar.dma_start(out=bias_sb, in_=bias_d[:, :])

        for win in range(NWIN):
            if win == 0:
                xwin = xwin0
            else:
                xwin = xpool.tile([P, KT, WIN], F32R, tag="xw", name="xwin")
                nc.sync.dma_start(
                    out=xwin, in_=xt_r[:, :, win * WIN:(win + 1) * WIN])
            for ss in range(WIN // P):
                s_lo = ss * P
                out_sb = opool.tile([P, D_OUT], F32, tag="o", name="out_sb")
                pos = [ps_o.tile([P, 512], F32, tag=f"po{jh}", name=f"po{jh}")
                       for jh in range(2)]
                for jh in range(2):
                    for kt in range(KT):
                        nc.tensor.matmul(
                            pos[jh],
                            xwin[:, kt, s_lo:s_lo + P],
                            w_tiles[kt][:, jh * 512:(jh + 1) * 512],
                            start=(kt == 0), stop=(kt == KT - 1))
                for jh in range(2):
                    nc.vector.tensor_add(
                        out=out_sb[:, jh * 512:(jh + 1) * 512],
                        in0=pos[jh],
                        in1=bias_sb[:, jh * 512:(jh + 1) * 512])
                st = win * (WIN // P) + ss
                nc.scalar.dma_start(
                    out=out_d[st * P:(st + 1) * P, :], in_=out_sb)

    nc.finalize()
    _NC_CACHE["nc"] = nc
    return nc


def _run_device(x, W_effT, bias_bcast, trace=False, tmpdir=None):
    """Run the SPMD kernel on 8 cores in this process. Returns (out, exec_ns)."""
    if _TRN_REPO not in sys.path:
        sys.path.insert(0, _TRN_REPO)
    from concourse.bass_utils import run_bass_kernel_spmd

    nc = _build_nc()
    core_ids = list(range(N_CORES))
    in_maps = [
        {"xt": np.ascontiguousarray(np.asarray(x[c], dtype=np.float32).T),
         "wt": W_effT, "bias": bias_bcast}
        for c in core_ids
    ]
    res = run_bass_kernel_spmd(nc, in_maps, core_ids, trace=trace, tmpdir=tmpdir)
    out = np.stack([res.results[c]["out"] for c in core_ids], axis=0)
    return out, res.exec_time_ns


def _kernel_impl(x, in_perm, out_perm, weight_blocks, brow, bcol, bias,
                 trace=False, tmpdir=None):
    x = np.asarray(x)
    W_effT, bias_bcast = _fold_weights(
        weight_blocks, brow, bcol, bias, in_perm, out_perm)
    out, exec_ns = _run_device(
        x.reshape(BATCH, SEQ, D_IN), W_effT, bias_bcast,
        trace=trace, tmpdir=tmpdir)
    return out.astype(np.float32), exec_ns


def _axon_usable_inproc():
    """True if this process can (still) drive the axon trn2 backend.
    If the caller pinned JAX_PLATFORMS to something without axon, importing
    jax here would initialize the wrong backend — run in a subprocess
    instead (and never half-initialize an axon client we can't use)."""
    jp = os.environ.get("JAX_PLATFORMS", "")
    if jp and "axon" not in jp:
        return False
    try:
        import jax
        return any(d.platform == "axon" for d in jax.devices())
    except Exception:
        return False


def kernel(x, in_perm, out_perm, weight_blocks, brow, bcol, bias):
    if _axon_usable_inproc():
        try:
            out, _ = _kernel_impl(
                x, in_perm, out_perm, weight_blocks, brow, bcol, bias)
            return out
        except Exception:
            pass
    # Fallback: run the device part in a clean subprocess (e.g. if the
    # calling process pinned JAX_PLATFORMS=cpu before importing jax).
    return _kernel_subprocess(
        x, in_perm, out_perm, weight_blocks, brow, bcol, bias)


def _kernel_subprocess(x, in_perm, out_perm, weight_blocks, brow, bcol, bias):
    with tempfile.TemporaryDirectory() as td:
        inp = os.path.join(td, "in.npz")
        outp = os.path.join(td, "out.npy")
        np.savez(inp, x=x, in_perm=in_perm, out_perm=out_perm,
                 weight_blocks=weight_blocks, brow=brow, bcol=bcol, bias=bias)
        env = dict(os.environ)
        env.pop("JAX_PLATFORMS", None)
        last_err = None
        for attempt in range(3):
            try:
                subprocess.run(
                    [sys.executable, os.path.abspath(__file__),
                     "--serve", inp, outp],
                    check=True, env=env)
                return np.load(outp)
            except subprocess.CalledProcessError as e:
                last_err = e
                import time
                time.sleep(15)
        raise last_err


if __name__ == "__main__":
    if len(sys.argv) == 4 and sys.argv[1] == "--serve":
        data = np.load(sys.argv[2])
        out, _ = _kernel_impl(
            data["x"], data["in_perm"], data["out_perm"],
            data["weight_blocks"], data["brow"], data["bcol"], data["bias"])
        np.save(sys.argv[3], out)


# revision 17
# speedup vs baseline: 1.0415x; 1.0025x over previous
"""BlockSparseLinearWithPerm Trainium2 kernel.

Math: out[b,s,j] = sum_i x[b,s,in_perm[i]] * W[out_perm[j], i] + bias[out_perm[j]]
where W is the dense form of the block-sparse weight.

Both permutations and the block scatter are folded on the host into a dense
effective weight  W_effT[k, j] = sum_{i: in_perm[i]==k} W[out_perm[j], i]
(host cost: one 1024x1024 scatter-add — negligible), so the device kernel is a
pure dense matmul  out = x @ W_effT + bias_eff, data-parallel over the batch
dim: one batch element (8192x1024 tokens) per NeuronCore, weights replicated.

Sharding/layout: each core's x slice is shipped feature-major (x^T) so the
contraction dim lands on SBUF partitions directly — the device spends zero
TensorE cycles on transposes and runs a pure f32r matmul stream (full
1 cycle/row PE rate, tf32-class mantissa, ~1e-3 scale-relative error).
Per 128-token tile: 16 accumulating f32r matmuls (lhsT = x^T k-tiles,
moving = resident W_effT) -> VectorE adds bias while copying PSUM -> SBUF
-> DMA out in natural token-major layout.
"""
import os
import sys
import subprocess
import tempfile

import numpy as np

_TRN_REPO = "/opt/trn_rl_repo"

D_IN = 1024
D_OUT = 1024
BS = 64
R = D_OUT // BS
C = D_IN // BS
BATCH = 8
SEQ = 8192
P = 128
KT = D_IN // P          # 8 k-tiles
WIN = 1024              # tokens per x^T window
NWIN = SEQ // WIN       # 8 windows
N_CORES = 8


def _fold_weights(weight_blocks, brow, bcol, bias, in_perm, out_perm):
    """Fold block scatter + both permutations into W_effT [k, j] and bias_eff."""
    wb = np.asarray(weight_blocks, dtype=np.float64)
    brow = np.asarray(brow).astype(np.int64)
    bcol = np.asarray(bcol).astype(np.int64)
    in_perm = np.asarray(in_perm).astype(np.int64)
    out_perm = np.asarray(out_perm).astype(np.int64)
    W4 = np.zeros((R, C, BS, BS), dtype=np.float64)
    W4[brow, bcol] = wb
    W = W4.transpose(0, 2, 1, 3).reshape(D_OUT, D_IN)
    Wp = W[out_perm]                       # [j, i]
    W_effT = np.zeros((D_IN, D_OUT), dtype=np.float64)
    np.add.at(W_effT, in_perm, Wp.T)       # row i of Wp.T added into row in_perm[i]
    bias_eff = np.asarray(bias, dtype=np.float64)[out_perm]
    bias_bcast = np.broadcast_to(bias_eff, (P, D_OUT)).copy()
    return (np.ascontiguousarray(W_effT, dtype=np.float64).astype(np.float32),
            bias_bcast.astype(np.float32))


_NC_CACHE = {}


def _build_nc():
    if "nc" in _NC_CACHE:
        return _NC_CACHE["nc"]
    if _TRN_REPO not in sys.path:
        sys.path.insert(0, _TRN_REPO)
    import concourse.bacc as bacc
    import concourse.mybir as mybir
    from concourse.tile import TileContext
    from contextlib import ExitStack

    F32 = mybir.dt.float32
    F32R = mybir.dt.float32r

    nc = bacc.Bacc(target_bir_lowering=False)
    xt_d = nc.declare_dram_parameter("xt", [D_IN, SEQ], F32R, isOutput=False)
    wt_d = nc.declare_dram_parameter("wt", [D_IN, D_OUT], F32R, isOutput=False)
    bias_d = nc.declare_dram_parameter("bias", [P, D_OUT], F32, isOutput=False)
    out_d = nc.declare_dram_parameter("out", [SEQ, D_OUT], F32, isOutput=True)

    xt_r = xt_d.rearrange("(kt p) s -> p kt s", p=P)
    wt_r = wt_d.rearrange("(kt p) j -> p kt j", p=P)

    with TileContext(nc) as tc, ExitStack() as ctx:
        consts = ctx.enter_context(tc.tile_pool(name="consts", bufs=1))
        xpool = ctx.enter_context(tc.tile_pool(name="xpool", bufs=2))
        opool = ctx.enter_context(tc.tile_pool(name="opool", bufs=4))
        ps_o = ctx.enter_context(tc.tile_pool(name="ps_o", bufs=4, space="PSUM"))

        bias_sb = consts.tile([P, D_OUT], F32)
        # Per-kt W tiles; x window chunks + W interleave across the two HWDGE
        # rings so the first matmul chains unblock early.
        w_tiles = [consts.tile([P, D_OUT], F32R, name=f"w_{kt}")
                   for kt in range(KT)]

        xwin0 = xpool.tile([P, KT, WIN], F32R, tag="xw", name="xwin")
        for kt in range(KT):
            nc.sync.dma_start(out=xwin0[:, kt], in_=xt_r[:, kt, 0:WIN])
            nc.scalar.dma_start(out=w_tiles[kt], in_=wt_r[:, kt])
        nc.scalar.dma_start(out=bias_sb, in_=bias_d[:, :])

        for win in range(NWIN):
            if win == 0:
                xwin = xwin0
            else:
                xwin = xpool.tile([P, KT, WIN], F32R, tag="xw", name="xwin")
                nc.sync.dma_start(
                    out=xwin, in_=xt_r[:, :, win * WIN:(win + 1) * WIN])
            for ss in range(WIN // P):
                s_lo = ss * P
                out_sb = opool.tile([P, D_OUT], F32, tag="o", name="out_sb")
                pos = [ps_o.tile([P, 512], F32, tag=f"po{jh}", name=f"po{jh}")
                       for jh in range(2)]
                st = win * (WIN // P) + ss
                for jh in range(2):
                    for kt in range(KT):
                        nc.tensor.matmul(
                            pos[jh],
                            xwin[:, kt, s_lo:s_lo + P],
                            w_tiles[kt][:, jh * 512:(jh + 1) * 512],
                            start=(kt == 0), stop=(kt == KT - 1))
                    # bias-add + store of this j-half streams while the other
                    # half's matmul chain is still on the PE
                    nc.vector.tensor_add(
                        out=out_sb[:, jh * 512:(jh + 1) * 512],
                        in0=pos[jh],
                        in1=bias_sb[:, jh * 512:(jh + 1) * 512])
                    nc.scalar.dma_start(
                        out=out_d[st * P:(st + 1) * P, jh * 512:(jh + 1) * 512],
                        in_=out_sb[:, jh * 512:(jh + 1) * 512])

    nc.finalize()
    _NC_CACHE["nc"] = nc
    return nc


def _run_device(x, W_effT, bias_bcast, trace=False, tmpdir=None):
    """Run the SPMD kernel on 8 cores in this process. Returns (out, exec_ns)."""
    if _TRN_REPO not in sys.path:
        sys.path.insert(0, _TRN_REPO)
    from concourse.bass_utils import run_bass_kernel_spmd

    nc = _build_nc()
    core_ids = list(range(N_CORES))
    in_maps = [
        {"xt": np.ascontiguousarray(np.asarray(x[c], dtype=np.float32).T),
         "wt": W_effT, "bias": bias_bcast}
        for c in core_ids
    ]
    res = run_bass_kernel_spmd(nc, in_maps, core_ids, trace=trace, tmpdir=tmpdir)
    out = np.stack([res.results[c]["out"] for c in core_ids], axis=0)
    return out, res.exec_time_ns


def _kernel_impl(x, in_perm, out_perm, weight_blocks, brow, bcol, bias,
                 trace=False, tmpdir=None):
    x = np.asarray(x)
    W_effT, bias_bcast = _fold_weights(
        weight_blocks, brow, bcol, bias, in_perm, out_perm)
    out, exec_ns = _run_device(
        x.reshape(BATCH, SEQ, D_IN), W_effT, bias_bcast,
        trace=trace, tmpdir=tmpdir)
    return out.astype(np.float32), exec_ns


def _axon_usable_inproc():
    """True if this process can (still) drive the axon trn2 backend.
    If the caller pinned JAX_PLATFORMS to something without axon, importing
    jax here would initialize the wrong backend — run in a subprocess
    instead (and never half-initialize an axon client we can't use)."""
    jp = os.environ.get("JAX_PLATFORMS", "")
    if jp and "axon" not in jp:
        return False
    try:
        import jax
        return any(d.platform == "axon" for d in jax.devices())
    except Exception:
        return False


def kernel(x, in_perm, out_perm, weight_blocks, brow, bcol, bias):
    if _axon_usable_inproc():
        try:
            out, _ = _kernel_impl(
                x, in_perm, out_perm, weight_blocks, brow, bcol, bias)
            return out
        except Exception:
            pass
    # Fallback: run the device part in a clean subprocess (e.g. if the
    # calling process pinned JAX_PLATFORMS=cpu before importing jax).
    return _kernel_subprocess(
        x, in_perm, out_perm, weight_blocks, brow, bcol, bias)


def _kernel_subprocess(x, in_perm, out_perm, weight_blocks, brow, bcol, bias):
    with tempfile.TemporaryDirectory() as td:
        inp = os.path.join(td, "in.npz")
        outp = os.path.join(td, "out.npy")
        np.savez(inp, x=x, in_perm=in_perm, out_perm=out_perm,
                 weight_blocks=weight_blocks, brow=brow, bcol=bcol, bias=bias)
        env = dict(os.environ)
        env.pop("JAX_PLATFORMS", None)
        last_err = None
        for attempt in range(3):
            try:
                subprocess.run(
                    [sys.executable, os.path.abspath(__file__),
                     "--serve", inp, outp],
                    check=True, env=env)
                return np.load(outp)
            except subprocess.CalledProcessError as e:
                last_err = e
                import time
                time.sleep(15)
        raise last_err


if __name__ == "__main__":
    if len(sys.argv) == 4 and sys.argv[1] == "--serve":
        data = np.load(sys.argv[2])
        out, _ = _kernel_impl(
            data["x"], data["in_perm"], data["out_perm"],
            data["weight_blocks"], data["brow"], data["bcol"], data["bias"])
        np.save(sys.argv[3], out)
